# revision 8
# baseline (speedup 1.0000x reference)
"""Trainium2 Bass kernel for nn_Encoder_33998961115156 (DGCNN-style encoder).

Sharding: data-parallel over batch B=16 across 8 NeuronCores (2 samples/core).
Per sample on device: 3x exact kNN (k=16) via DVE max8/max_index/match_replace,
neighbor gathers via indirect DMA from DRAM row tables, dense layers on PE
(BN folded into weights on host), fused global max-pool + MLP head.

Self-contained: hardcodes all shapes; needs only /opt/trn_rl_repo (concourse),
numpy and jax (for device execution via PJRT).
"""
import sys

sys.path.insert(0, "/opt/trn_rl_repo")

import numpy as np

import concourse.bass as bass
import concourse.bacc as bacc
import concourse.mybir as mybir
from concourse.tile import TileContext
from concourse.masks import make_identity

N = 4096
K = 16
NT = N // 128
S_PER_CORE = 2
BN_EPS = 1e-3
F32 = mybir.dt.float32
U32 = mybir.dt.uint32
ACT = mybir.ActivationFunctionType
OP = mybir.AluOpType

_RUNNER = None


def build_device_program(reps=1, stop_after=None):
    nc = bacc.Bacc("TRN2", target_bir_lowering=False, debug=False)

    xT_d = nc.dram_tensor("xT", [S_PER_CORE, 3, N], F32, kind="ExternalInput")
    xrows_d = [
        nc.dram_tensor(f"xrows{s}", [N, 3], F32, kind="ExternalInput")
        for s in range(S_PER_CORE)
    ]
    w_shapes = [
        ("w1", [12, 12]), ("b1", [12, 1]),
        ("w2", [12, 64]), ("b2", [64, 1]),
        ("w3", [64, 64]), ("b3", [64, 1]),
        ("wg1a", [64, 64]), ("bg1a", [64, 1]),
        ("wg1b", [64, 128]), ("bg1b", [128, 1]),
        ("wg2a", [128, 128]), ("bg2a", [128, 1]),
        ("wg2b", [128, 1024]), ("bg2b", [128, 8]),
        ("b4", [128, 8]), ("b5", [128, 4]),
    ]
    w_d = {nm: nc.dram_tensor(nm, sh, F32, kind="ExternalInput") for nm, sh in w_shapes}
    w4_d = nc.dram_tensor("w4", [128, 8192], F32, kind="ExternalInput")
    w5_d = nc.dram_tensor("w5", [128, 4096], F32, kind="ExternalInput")
    out_d = nc.dram_tensor("out", [S_PER_CORE, 512], F32, kind="ExternalOutput")
    h1rows_d = nc.dram_tensor("h1rows", [N, 64], F32)
    h2rows_d = nc.dram_tensor("h2rows", [N, 128], F32)

    with TileContext(nc) as tc:
        with (
            tc.tile_pool(name="const", bufs=1) as cpool,
            tc.tile_pool(name="wpool", bufs=1) as wpool,
            tc.tile_pool(name="hpool", bufs=1) as hpool,
            tc.tile_pool(name="sb", bufs=2) as sb,
            tc.tile_pool(name="aux", bufs=1) as auxpool,
            tc.tile_pool(name="ps", bufs=2, space="PSUM") as pspool,
        ):
            # ---- constants / weights ----
            ident = cpool.tile([128, 128], F32)
            make_identity(nc, ident[:])
            ones_t = cpool.tile([1, N], F32)
            nc.gpsimd.memset(ones_t[:], 1.0)
            wsb = {}
            for nm, sh in w_shapes:
                t = wpool.tile(sh, F32, tag=f"w_{nm}")
                nc.sync.dma_start(t[:], w_d[nm][:])
                wsb[nm] = t

            def dense_chain(w, b, src, dst, func):
                cout = w.shape[1]
                for j0 in range(0, N, 512):
                    ps = pspool.tile([128, 512], F32, tag="ps")
                    nc.tensor.matmul(
                        ps[:cout, :], lhsT=w[:], rhs=src[:, j0 : j0 + 512],
                        start=True, stop=True,
                    )
                    nc.scalar.activation(
                        dst[:, j0 : j0 + 512], ps[:cout, :], func, bias=b[:]
                    )

            def build_aux(hT, C, aux):
                """aux = -|h_j|^2/2 row [1, N], via Square + (-0.5)-ones matmul."""
                hsq = sb.tile([128, N], F32, tag="ndc")
                nc.scalar.activation(hsq[:C, :], hT[:, :], ACT.Square)
                negh = cpool.tile([128, 1], F32, tag="negh")
                nc.vector.memset(negh[:], -0.5)
                for j0 in range(0, N, 512):
                    ps = pspool.tile([128, 512], F32, tag="ps")
                    nc.tensor.matmul(
                        ps[:1, :], lhsT=negh[:C, :], rhs=hsq[:C, j0 : j0 + 512],
                        start=True, stop=True,
                    )
                    nc.scalar.activation(aux[0:1, j0 : j0 + 512], ps[:1, :], ACT.Copy)

            def knn_tiles(C, hT, aux, rows_dram, idx_offset, out_cb):
                """Per 128-query tile: nd, exact top-16, gather, out_cb(I, gk, idx)."""
                for I in range(NT):
                    i0 = I * 128
                    ndc = sb.tile([128, N], F32, tag="ndc")
                    for h0 in range(0, N, 2048):
                        nd = pspool.tile([128, 2048], F32, tag="ps")
                        for j0 in range(0, 2048, 512):
                            nc.tensor.matmul(
                                nd[:, j0 : j0 + 512],
                                lhsT=hT[:, i0 : i0 + 128],
                                rhs=hT[:, h0 + j0 : h0 + j0 + 512],
                                start=True, stop=False,
                            )
                            nc.tensor.matmul(
                                nd[:, j0 : j0 + 512],
                                lhsT=ones_t[:, i0 : i0 + 128],
                                rhs=aux[:, h0 + j0 : h0 + j0 + 512],
                                start=False, stop=True,
                            )
                        nc.scalar.activation(
                            ndc[:, h0 : h0 + 2048], nd[:], ACT.Copy
                        )
                    m8a = sb.tile([128, 8], F32, tag="m8a")
                    m8b = sb.tile([128, 8], F32, tag="m8b")
                    idx = sb.tile([128, K], U32, tag="idx")
                    nc.vector.max(out=m8a[:], in_=ndc[:])
                    nc.vector.max_index(out=idx[:, 0:8], in_max=m8a[:], in_values=ndc[:])
                    nc.vector.match_replace(
                        out=ndc[:], in_to_replace=m8a[:], in_values=ndc[:],
                        imm_value=-1e30,
                    )
                    nc.vector.max(out=m8b[:], in_=ndc[:])
                    nc.vector.max_index(
                        out=idx[:, 8:16], in_max=m8b[:], in_values=ndc[:]
                    )
                    if idx_offset:
                        nc.vector.tensor_scalar(
                            out=idx[:], in0=idx[:], scalar1=idx_offset, scalar2=None,
                            op0=OP.add,
                        )
                    gk = sb.tile([128, K * C], F32, tag="gk")
                    for k in range(K):
                        nc.gpsimd.indirect_dma_start(
                            out=gk[:, k * C : (k + 1) * C],
                            out_offset=None,
                            in_=rows_dram[:],
                            in_offset=bass.IndirectOffsetOnAxis(
                                ap=idx[:, k : k + 1], axis=0
                            ),
                        )
                    out_cb(I, gk, idx)

            def kmax_transpose(gk, C, dstT, i0):
                """max over K of gk [128, K*C] then transpose into dstT[:, i0:+128]."""
                cur = gk
                width = K
                while width > 1:
                    half = width // 2
                    nxt = sb.tile([128, half * C], F32, tag=f"fold{C}_{half}")
                    nc.vector.tensor_tensor(
                        out=nxt[:], in0=cur[:, : half * C],
                        in1=cur[:, half * C : width * C], op=OP.max,
                    )
                    cur = nxt
                    width = half
                tp = pspool.tile([C, 128], F32, tag="ps")
                nc.tensor.transpose(tp[:], cur[:], ident[:])
                nc.scalar.activation(dstT[:, i0 : i0 + 128], tp[:], ACT.Copy)

            for rep in range(reps):
                for s in range(S_PER_CORE):
                    # ============ phase A: xyz knn + covariance features =====
                    xt = hpool.tile([3, N], F32, tag="hd")
                    nc.sync.dma_start(xt[:], xT_d[s])
                    aux = auxpool.tile([1, N], F32, tag="aux")
                    build_aux(xt, 3, aux)
                    h0T = hpool.tile([12, N], F32, tag="ha")

                    def cov_cb(I, gk, idx, s=s, h0T=h0T):
                        i0 = I * 128
                        xsum = sb.tile([128, 3], F32, tag="xsum")
                        gk_ck = bass.AP(
                            gk.tensor, gk.offset, [gk.ap[0], [1, 3], [3, K]]
                        )
                        nc.vector.tensor_reduce(
                            out=xsum[:], in_=gk_ck, axis=mybir.AxisListType.X,
                            op=OP.add,
                        )
                        mean = sb.tile([128, 3], F32, tag="mean")
                        nc.scalar.activation(mean[:], xsum[:], ACT.Copy, scale=1.0 / K)
                        xc = sb.tile([128, K * 3], F32, tag="xc")
                        mean_b = bass.AP(
                            mean.tensor, mean.offset, [mean.ap[0], [0, K], [1, 3]]
                        )
                        nc.vector.tensor_tensor(
                            out=xc[:], in0=gk[:], in1=mean_b, op=OP.subtract
                        )
                        prod = sb.tile([128, 144], F32, tag="prod")
                        xc_i = bass.AP(
                            xc.tensor, xc.offset, [xc.ap[0], [1, 3], [0, 3], [3, K]]
                        )
                        xc_j = bass.AP(
                            xc.tensor, xc.offset, [xc.ap[0], [0, 3], [1, 3], [3, K]]
                        )
                        prod_v = bass.AP(
                            prod.tensor, prod.offset,
                            [prod.ap[0], [16 * 3, 3], [16, 3], [1, K]],
                        )
                        nc.vector.tensor_tensor(
                            out=prod_v, in0=xc_i, in1=xc_j, op=OP.mult
                        )
                        feat = sb.tile([128, 12], F32, tag="feat")
                        nc.sync.dma_start(feat[:, 0:3], xrows_d[s][i0 : i0 + 128])
                        prod_r = bass.AP(
                            prod.tensor, prod.offset, [prod.ap[0], [16, 9], [1, K]]
                        )
                        nc.vector.tensor_reduce(
                            out=feat[:, 3:12], in_=prod_r, axis=mybir.AxisListType.X,
                            op=OP.add,
                        )
                        tp = pspool.tile([12, 128], F32, tag="ps")
                        nc.tensor.transpose(tp[:], feat[:], ident[:])
                        nc.scalar.activation(h0T[:, i0 : i0 + 128], tp[:], ACT.Copy)

                    knn_tiles(3, xt, aux, xrows_d[s], 0, cov_cb)
                    if stop_after == "A":
                        nc.sync.dma_start(
                            bass.AP(out_d, s * 512, [[1, 2], [2, 256]]),
                            h0T[0:2, 0:256],
                        )
                        continue

                    # ============ phase B: conv stack 12->12->64->64 ==========
                    t1 = hpool.tile([12, N], F32, tag="hb")
                    t2 = hpool.tile([64, N], F32, tag="hc")
                    h1T = hpool.tile([64, N], F32, tag="hd")
                    dense_chain(wsb["w1"], wsb["b1"], h0T, t1, ACT.Relu)
                    dense_chain(wsb["w2"], wsb["b2"], t1, t2, ACT.Relu)
                    dense_chain(wsb["w3"], wsb["b3"], t2, h1T, ACT.Relu)
                    if stop_after == "B0":
                        nc.sync.dma_start(
                            bass.AP(out_d, s * 512, [[1, 2], [2, 256]]),
                            h1T[0:2, 0:256],
                        )
                        continue
                    for I in range(NT):
                        i0 = I * 128
                        tp = pspool.tile([128, 64], F32, tag="ps")
                        nc.tensor.transpose(
                            tp[:], h1T[:, i0 : i0 + 128], ident[:64, :64]
                        )
                        rc = sb.tile([128, 128], F32, tag="rc")
                        nc.scalar.activation(rc[:, :64], tp[:], ACT.Copy)
                        nc.sync.dma_start(h1rows_d[i0 : i0 + 128], rc[:, :64])
                    if stop_after == "B":
                        nc.sync.dma_start(
                            bass.AP(out_d, s * 512, [[1, 2], [2, 256]]),
                            h1T[0:2, 0:256],
                        )
                        continue

                    # ============ phase C: graph layer 1 ======================
                    aux = auxpool.tile([1, N], F32, tag="aux")
                    build_aux(h1T, 64, aux)
                    g1T = hpool.tile([64, N], F32, tag="ha")

                    def g1_cb(I, gk, idx, g1T=g1T):
                        kmax_transpose(gk, 64, g1T, I * 128)

                    knn_tiles(64, h1T, aux, h1rows_d, 0, g1_cb)

                    if stop_after == "C0":
                        nc.sync.dma_start(
                            bass.AP(out_d, s * 512, [[1, 2], [2, 256]]),
                            g1T[0:2, 0:256],
                        )
                        continue
                    tg = hpool.tile([64, N], F32, tag="hb")
                    h2T = hpool.tile([128, N], F32, tag="hc")
                    dense_chain(wsb["wg1a"], wsb["bg1a"], g1T, tg, ACT.Identity)
                    dense_chain(wsb["wg1b"], wsb["bg1b"], tg, h2T, ACT.Relu)
                    for I in range(NT):
                        i0 = I * 128
                        tp = pspool.tile([128, 128], F32, tag="ps")
                        nc.tensor.transpose(tp[:], h2T[:, i0 : i0 + 128], ident[:])
                        rc = sb.tile([128, 128], F32, tag="rc")
                        nc.scalar.activation(rc[:], tp[:], ACT.Copy)
                        nc.sync.dma_start(h2rows_d[i0 : i0 + 128], rc[:])

                    # ============ phase D: graph layer 2 + head ===============
                    aux = auxpool.tile([1, N], F32, tag="aux")
                    build_aux(h2T, 128, aux)
                    g2T = hpool.tile([128, N], F32, tag="ha")

                    def g2_cb(I, gk, idx, g2T=g2T):
                        kmax_transpose(gk, 128, g2T, I * 128)

                    knn_tiles(128, h2T, aux, h2rows_d, 0, g2_cb)

                    if stop_after == "D0":
                        nc.sync.dma_start(
                            bass.AP(out_d, s * 512, [[1, 2], [2, 256]]),
                            g2T[0:2, 0:256],
                        )
                        continue
                    tg2 = hpool.tile([128, N], F32, tag="hc")
                    dense_chain(wsb["wg2a"], wsb["bg2a"], g2T, tg2, ACT.Identity)
                    if stop_after == "D1":
                        nc.sync.dma_start(
                            bass.AP(out_d, s * 512, [[1, 2], [2, 256]]),
                            tg2[0:2, 0:256],
                        )
                        continue
                    # gl2_conv 128->1024 fused with global max over points
                    h3 = auxpool.tile([128, 8], F32, tag="h3")
                    rmax = sb.tile([128, 1], F32, tag="rmax")
                    for og in range(8):
                        for j0 in range(0, N, 512):
                            ps = pspool.tile([128, 512], F32, tag="ps")
                            nc.tensor.matmul(
                                ps[:],
                                lhsT=wsb["wg2b"][:, og * 128 : (og + 1) * 128],
                                rhs=tg2[:, j0 : j0 + 512],
                                start=True, stop=True,
                            )
                            if j0 == 0:
                                nc.vector.tensor_reduce(
                                    out=h3[:, og : og + 1], in_=ps[:],
                                    axis=mybir.AxisListType.X, op=OP.max,
                                )
                            else:
                                nc.vector.tensor_reduce(
                                    out=rmax[:], in_=ps[:],
                                    axis=mybir.AxisListType.X, op=OP.max,
                                )
                                nc.vector.tensor_tensor(
                                    out=h3[:, og : og + 1],
                                    in0=h3[:, og : og + 1], in1=rmax[:], op=OP.max,
                                )
                    nc.vector.tensor_tensor(
                        out=h3[:], in0=h3[:], in1=wsb["bg2b"][:], op=OP.add
                    )
                    if stop_after == "D2":
                        nc.sync.dma_start(
                            bass.AP(out_d, s * 512, [[1, 128], [128, 4]]),
                            h3[:, 0:4],
                        )
                        continue
                    # conv4 (relu) then conv5, channels-on-partitions layout
                    h4 = auxpool.tile([128, 8], F32, tag="h4")
                    for oc in range(8):
                        wh = sb.tile([128, 1024], F32, tag="wh")
                        w4_ap = bass.AP(
                            w4_d, oc * 128, [[8192, 128], [1024, 8], [1, 128]]
                        )
                        nc.sync.dma_start(wh[:].rearrange("p (a b) -> p a b", a=8), w4_ap)
                        ps = pspool.tile([128, 1], F32, tag="ps")
                        for ic in range(8):
                            nc.tensor.matmul(
                                ps[:], lhsT=wh[:, ic * 128 : (ic + 1) * 128],
                                rhs=h3[:, ic : ic + 1],
                                start=(ic == 0), stop=(ic == 7),
                            )
                        nc.scalar.activation(
                            h4[:, oc : oc + 1], ps[:], ACT.Relu,
                            bias=wsb["b4"][:, oc : oc + 1],
                        )
                    if stop_after == "D3":
                        nc.sync.dma_start(
                            bass.AP(out_d, s * 512, [[1, 128], [128, 4]]),
                            h4[:, 0:4],
                        )
                        continue
                    h5 = auxpool.tile([128, 4], F32, tag="h5")
                    for oc in range(4):
                        wh = sb.tile([128, 1024], F32, tag="wh")
                        w5_ap = bass.AP(
                            w5_d, oc * 128, [[4096, 128], [512, 8], [1, 128]]
                        )
                        nc.sync.dma_start(wh[:].rearrange("p (a b) -> p a b", a=8), w5_ap)
                        ps = pspool.tile([128, 1], F32, tag="ps")
                        for ic in range(8):
                            nc.tensor.matmul(
                                ps[:], lhsT=wh[:, ic * 128 : (ic + 1) * 128],
                                rhs=h4[:, ic : ic + 1],
                                start=(ic == 0), stop=(ic == 7),
                            )
                        nc.scalar.activation(
                            h5[:, oc : oc + 1], ps[:], ACT.Identity,
                            bias=wsb["b5"][:, oc : oc + 1],
                        )
                    out_ap = bass.AP(out_d, s * 512, [[1, 128], [128, 4]])
                    nc.sync.dma_start(out_ap, h5[:])

    nc.compile()
    return nc


# ---------------------------------------------------------------- host wrapper
def _fold_bn(wd, bd, bn):
    s = np.asarray(bn["gamma"]) / np.sqrt(np.asarray(bn["var"]) + BN_EPS)
    return (
        np.asarray(wd) * s[None, :],
        (np.asarray(bd) - np.asarray(bn["mean"])) * s + np.asarray(bn["beta"]),
    )


def _prep_weights(params):
    p = {k: {kk: np.asarray(vv) for kk, vv in v.items()} for k, v in params.items()}
    w1, b1 = _fold_bn(p["conv1"]["w"], p["conv1"]["b"], p["bn1"])
    w2, b2 = _fold_bn(p["conv2"]["w"], p["conv2"]["b"], p["bn2"])
    w3, b3 = _fold_bn(p["conv3"]["w"], p["conv3"]["b"], p["bn3"])
    w4 = p["conv4"]["w"].reshape(8, 128, 1024).transpose(1, 0, 2).reshape(128, 8192)
    w5 = p["conv5"]["w"].reshape(8, 128, 512).transpose(1, 0, 2).reshape(128, 4096)
    d = {
        "w1": w1, "b1": b1[:, None],
        "w2": w2, "b2": b2[:, None],
        "w3": w3, "b3": b3[:, None],
        "wg1a": p["gl1_lin"]["w"], "bg1a": p["gl1_lin"]["b"][:, None],
        "wg1b": p["gl1_conv"]["w"], "bg1b": p["gl1_conv"]["b"][:, None],
        "wg2a": p["gl2_lin"]["w"], "bg2a": p["gl2_lin"]["b"][:, None],
        "wg2b": p["gl2_conv"]["w"],
        "bg2b": p["gl2_conv"]["b"].reshape(8, 128).T,
        "w4": w4, "b4": p["conv4"]["b"].reshape(8, 128).T,
        "w5": w5, "b5": p["conv5"]["b"].reshape(4, 128).T,
    }
    return {k: np.ascontiguousarray(v, np.float32) for k, v in d.items()}


def make_in_maps(x, params):
    x = np.asarray(x, np.float32)
    wd = _prep_weights(params)
    in_maps = []
    for c in range(8):
        xs = x[c * 2 : (c + 1) * 2]
        m = dict(wd)
        m["xT"] = np.ascontiguousarray(xs.transpose(0, 2, 1))
        m["xrows0"] = np.ascontiguousarray(xs[0])
        m["xrows1"] = np.ascontiguousarray(xs[1])
        in_maps.append(m)
    return in_maps


def get_runner(reps=1):
    global _RUNNER
    if _RUNNER is None:
        from runner import build_runner

        nc = build_device_program(reps)
        _RUNNER = build_runner(nc, 8)
    return _RUNNER


def kernel(x, params):
    run = get_runner()
    res = run(make_in_maps(x, params))
    out = np.concatenate([r["out"] for r in res], axis=0)
    return out.reshape(16, 1, 512).astype(np.float32)
